# revision 86
# baseline (speedup 1.0000x reference)
"""Multi-head self-attention with RoPE + causal mask on 8 Trainium2 NeuronCores.

Sharding: batch x head hybrid. Core c owns batch c//2 and head-half c%2
(8 of the 16 heads, a 512-wide slice of the QKV output dim / Wo input dim).
Each core computes a partial output out_c = O_c @ Wo_c^T of one batch; the
host sums the 2 partials per batch (the Wo row-split all-reduce done
host-side at gather time).

All matmul operands are bf16 (fp32 PSUM accumulation). The PE cost per
matmul is its OUTPUT free size (rows), so the kernel is organized to
minimize total rows:
  - Q^T/K^T produced in [dims, S] layout; RoPE de-interleave folded into
    the weight rows host-side; the +-32-partition swap via a PE
    permutation matmul; cos/sin combine on DVE.
  - V stored per 128-seq chunk as [128 seq, 8 heads, 65]: 64 dims + a
    ones column so the softmax denominator falls out of the P.V matmul.
  - ScoresT[sk, sq] = K^T.T @ Q^T per head pair (shared PSUM tile,
    (0,0)/(64,0) row tiles, K=64). exp on ScalarE from PSUM (1/8 scale
    folded; inputs bounded so no max subtraction). Causal masking
    multiplies only the 128x128 diagonal triangles.
  - P.V is TRANSPOSED: stationary = exp'd score chunk [sk, sq-128],
    moving = V chunk [sk, 65] -> po[sq, 65] accumulated over sk chunks.
    65 output rows per matmul instead of the sq-tile width: all 128
    output partitions carry sq, halving PV row cost (the matmuls
    stream at ~31ns; the 128-row FastWeightLoad hides under them).
    PSUM start zeroes a whole 2KB bank lazily, so one start per
    hi-bank per pass lets the 4 sq-chunk accumulations share a bank.
    den lands in column 64; normalization is a per-partition
    tensor_scalar on DVE; the [sq, d] -> [d, sq] fixup is a PE
    transpose (128 rows) + DVE drain, deferred one pass so it never
    head-of-line-waits on the norm.
  - The PE queue is strictly in-order and the attention phase is
    ACT(exp)-bound, so emission order IS the schedule: PV groups are
    emitted one k-chunk late, and ALL projection work (each rep's Q/K
    groups with per-tile deadlines, V chunks with per-chunk deadlines,
    out-projections one tile late) is split into SINGLE-MATMUL filler
    units paced evenly over the rep's 160 score-chunk iterations --
    the PE is the global bottleneck (~93% busy), so spreading the
    whole supply keeps ACT fully hidden and leaves no dead blob at
    rep boundaries. SBUF-only RoPE combine ops run on the otherwise
    idle Pool engine (GPSIMD cannot access PSUM).
"""

import sys

sys.path.insert(0, "/opt/trn_rl_repo")

import numpy as np
from collections import deque
from contextlib import ExitStack

import concourse.bass as bass
import concourse.tile as tile
from concourse import bacc, mybir
from concourse.bass_utils import run_bass_kernel_spmd

F32 = mybir.dt.float32
BF16 = mybir.dt.bfloat16

# problem constants (hardcoded per harness contract)
B = 4
S = 2048
D = 1024
NUM_HEADS = 16
DK = 64
THETA = 10000.0
NCORES = 8
HPC = 8  # heads per core
BLK = HPC * DK  # 512-wide per-core head-dim block
NDC = BLK // 128  # 4 dim chunks of 128
P = 128
SQT = 512  # sq tile width
NKC = D // P  # 8 contraction chunks for the projections

# pacing model (ns)
ROW_NS = 0.4166  # PE ns per output row at full clock
ACT_NS = 0.8333  # ACT ns per element per partition
ACT_FIX = 217.0  # ACT per-instruction fixed cost
PV_MM_NS = 45.0  # per 65-row PV matmul (rows + partially hidden LDW)
PASS_BONUS = 600.0  # extra filler at pass start (po ring-1 WAR cover)
UNIT_NS = SQT * ROW_NS  # nominal per-filler-unit PE cost


def build_program(b=B, s=S, reps=1):
    """Build the (SPMD-shared) per-core Bass program.

    reps>1 repeats the whole computation (for marginal-cost timing)."""
    nc = bacc.Bacc("TRN2", target_bir_lowering=False, debug=False)

    n_sqt = s // SQT  # sq tiles
    n_skc = s // P  # seq chunks of 128

    # ---- DRAM I/O ----
    xT = nc.dram_tensor("xT", [NKC, P, s], BF16, kind="ExternalInput").ap()
    wqT = nc.dram_tensor("wqT", [NKC, P, BLK], BF16, kind="ExternalInput").ap()
    wkT = nc.dram_tensor("wkT", [NKC, P, BLK], BF16, kind="ExternalInput").ap()
    wvT = nc.dram_tensor("wvT", [NKC, P, BLK], BF16, kind="ExternalInput").ap()
    woT = nc.dram_tensor("woT", [NDC, P, D], BF16, kind="ExternalInput").ap()
    costab = nc.dram_tensor("costab", [P, s], BF16, kind="ExternalInput").ap()
    sintab = nc.dram_tensor("sintab", [P, s], BF16, kind="ExternalInput").ap()
    pmswap = nc.dram_tensor("pmswap", [P, P], BF16, kind="ExternalInput").ap()
    causal = nc.dram_tensor("causal", [P, P], BF16, kind="ExternalInput").ap()
    identt = nc.dram_tensor("identt", [P, P], BF16, kind="ExternalInput").ap()
    out = nc.dram_tensor("out", [s, D], BF16, kind="ExternalOutput").ap()

    FENCE = ("fence",)  # separates one rep's Q/K units from the next's

    with tile.TileContext(nc) as tc, ExitStack() as ctx:
        consts = ctx.enter_context(tc.tile_pool(name="consts", bufs=1))
        # xt tiles stay alive from the (interleaved, early) Q/K groups until
        # the V projections of the owning rep: ring of 5 gives the cross-rep
        # WAR an extra tile of slack.
        xpool = ctx.enter_context(tc.tile_pool(name="xpool", bufs=5))
        big = ctx.enter_context(tc.tile_pool(name="big", bufs=1))
        # qT/kT double-buffered: the next rep's Q/K groups (written during
        # this rep's attention) must not clobber the tiles attention reads.
        qkp = ctx.enter_context(tc.tile_pool(name="qkp", bufs=2))
        # ring 3: since the raw/V casts moved to ACT, a slot-WAR stall on a
        # Copy head-of-line blocks later exps in the ACT FIFO -- the extra
        # buffer keeps those Copies start-ready
        work = ctx.enter_context(tc.tile_pool(name="work", bufs=3))
        expp = ctx.enter_context(tc.tile_pool(name="expp", bufs=4))
        psum = ctx.enter_context(tc.tile_pool(name="psum", bufs=2, space="PSUM"))
        opsum = ctx.enter_context(tc.tile_pool(name="opsum", bufs=1, space="PSUM"))

        # ---- constants resident in SBUF ----
        w_sb = {}
        for name, ap in (("wq", wqT), ("wk", wkT), ("wv", wvT)):
            t = consts.tile([P, NKC, BLK], BF16, tag=f"w_{name}")
            for kc in range(NKC):
                nc.sync.dma_start(t[:, kc], ap[kc])
            w_sb[name] = t
        wo_sb = consts.tile([P, NDC, D], BF16, tag="wo")
        for dc in range(NDC):
            nc.sync.dma_start(wo_sb[:, dc], woT[dc])
        cos_sb = consts.tile([P, s], BF16, tag="cos")
        nc.sync.dma_start(cos_sb[:], costab)
        sin_sb = consts.tile([P, s], BF16, tag="sin")
        nc.sync.dma_start(sin_sb[:], sintab)
        pm_sb = consts.tile([P, P], BF16, tag="pm")
        nc.sync.dma_start(pm_sb[:], pmswap)
        ca_sb = consts.tile([P, P], BF16, tag="causal")
        nc.sync.dma_start(ca_sb[:], causal)
        id_sb = consts.tile([P, P], BF16, tag="ident")
        nc.sync.dma_start(id_sb[:], identt)

        class RepState:
            def __init__(self, rep_id=0):
                self.rep_id = rep_id
                self.qT = qkp.tile([P, NDC, s], BF16, tag="qT", name="qT")
                self.kT = qkp.tile([P, NDC, s], BF16, tag="kT", name="kT")
                self.oT = None  # allocated at attention time
                self.xts = {}  # t_i -> list of 8 xt tiles (shared Q/K/V)
                self.v_sb = None  # allocated at the first V unit

        def get_xt(st, t_i):
            if t_i not in st.xts:
                w = slice(t_i * SQT, (t_i + 1) * SQT)
                xt = []
                for kc in range(NKC):
                    xtc = xpool.tile([P, SQT], BF16, tag=f"xt{kc}", name=f"xt{kc}")
                    nc.sync.dma_start(xtc[:], xT[kc, :, w])
                    xt.append(xtc)
                st.xts[t_i] = xt
            return st.xts[t_i]

        # ---------- filler units: (pe_cost_ns, emit_fn) ----------

        def qk_group_units(st, t_i, name, dc):
            """One Q or K projection group as 9 single-matmul units."""
            w = slice(t_i * SQT, (t_i + 1) * SQT)
            dst = st.qT if name == "wq" else st.kT
            box = {}

            def mm(kc):
                def emit():
                    if kc == 0:
                        box["xt"] = get_xt(st, t_i)
                        box["ps"] = psum.tile([P, SQT], F32, tag="mm", name="ps")
                    nc.tensor.matmul(
                        box["ps"][:],
                        w_sb[name][:, kc, dc * P : (dc + 1) * P],
                        box["xt"][kc][:],
                        start=(kc == 0),
                        stop=(kc == NKC - 1),
                    )
                    if kc == NKC - 1:
                        raw = work.tile([P, SQT], BF16, tag="raw")
                        # ACT Copy (same act table as Exp, no switch cost):
                        # starts right when mm7 drains instead of queueing
                        # behind DVE filler work -- the swap matmul and the
                        # next group's mm-ring WAR both wait on this cast
                        nc.scalar.activation(
                            raw[:], box["ps"][:],
                            mybir.ActivationFunctionType.Copy,
                        )
                        box["raw"] = raw

                return emit

            def swap():
                raw = box["raw"]
                ps_sw = psum.tile([P, SQT], F32, tag="mm", name="ps_sw")
                nc.tensor.matmul(ps_sw[:], pm_sb[:], raw[:], start=True, stop=True)
                # tco and the final add are SBUF-only: run them on the idle
                # Pool engine, halving this chain's DVE queue footprint
                tco = work.tile([P, SQT], BF16, tag="tco")
                nc.gpsimd.tensor_tensor(
                    tco[:], raw[:], cos_sb[:, w], mybir.AluOpType.mult
                )
                tsi = work.tile([P, SQT], BF16, tag="tsi")
                nc.vector.tensor_tensor(
                    tsi[:], ps_sw[:], sin_sb[:, w], mybir.AluOpType.mult
                )
                nc.gpsimd.tensor_tensor(
                    dst[:, dc, w], tco[:], tsi[:], mybir.AluOpType.add
                )

            units = [(st.rep_id, t_i, SQT * ROW_NS, mm(kc)) for kc in range(NKC)]
            units.append((st.rep_id, t_i, SQT * ROW_NS, swap))
            return units

        def v_sc_units(st, t_i, sc8):
            """V projection for one 128-seq chunk as 8 units, tagged with the
            chunk index so PV emission can force-drain them in time."""
            if st.v_sb is None:
                st.v_sb = [
                    big.tile([P, HPC, 65], BF16, tag=f"v{sc}", name=f"v{sc}")
                    for sc in range(n_skc)
                ]
            sc = t_i * (SQT // P) + sc8
            box = {}

            def mm(kc):
                def emit():
                    if kc == 0:
                        box["xt"] = get_xt(st, t_i)
                        box["ps"] = psum.tile([P, SQT], F32, tag="mm", name="ps_v")
                    nc.tensor.matmul(
                        box["ps"][:],
                        box["xt"][kc][:, sc8 * P : (sc8 + 1) * P],
                        w_sb["wv"][:, kc],
                        start=(kc == 0),
                        stop=(kc == NKC - 1),
                    )
                    if kc == NKC - 1:
                        vt = st.v_sb[sc]
                        nc.gpsimd.memset(vt[:, :, DK:65], 1.0)
                        # ACT Copy (shares Exp's act table): starts when the
                        # projection drains; the PV moving read gates on it
                        nc.scalar.activation(
                            vt[:, :, 0:DK], box["ps"][:],
                            mybir.ActivationFunctionType.Copy,
                        )

                return emit

            return [(sc, SQT * ROW_NS, mm(kc)) for kc in range(NKC)]

        def proj_units(oT_src, st_c, nt):
            """One out-projection group (4 accumulating matmuls + drain)."""
            box = {}

            def mm(dc):
                def emit():
                    if dc == 0:
                        box["ps"] = psum.tile([P, SQT], F32, tag="mm", name="ps_p")
                    nc.tensor.matmul(
                        box["ps"][:],
                        oT_src[:, dc, st_c * P : (st_c + 1) * P],
                        wo_sb[:, dc, nt * SQT : (nt + 1) * SQT],
                        start=(dc == 0),
                        stop=(dc == NDC - 1),
                    )
                    if dc == NDC - 1:
                        ob = work.tile([P, SQT], BF16, tag="ob")
                        nc.vector.tensor_copy(ob[:], box["ps"][:])
                        nc.sync.dma_start(
                            out[
                                st_c * P : (st_c + 1) * P,
                                nt * SQT : (nt + 1) * SQT,
                            ],
                            ob[:],
                        )

                return emit

            return [(SQT * ROW_NS, mm(dc)) for dc in range(NDC)]

        # ---------- attention ----------

        def emit_attention(st, fq, vq, pq):
            """fq: deque of (cost, fn) filler units (next rep's Q/K groups).
            vq: deque of (chunk, cost, fn) V units with chunk deadlines,
            seeded with tile-0 chunks. pq: deque of out-projection units,
            seeded with the previous rep's last tile."""
            st.oT = big.tile([P, NDC, s], BF16, tag="oT", name="oT")
            qT, kT, oT = st.qT, st.kT, st.oT
            # pace the whole rep's filler supply (next rep's Q/K groups, V
            # chunks, out-projections) evenly over the 160 skc iterations:
            # the PE is the global bottleneck, so spreading ALL of it through
            # the ACT-bound attention keeps ACT fully hidden and leaves no
            # dead blob at the rep boundary.
            n_skc_total = sum(4 * ((t + 1) * SQT // P) for t in range(n_sqt))
            supply = (len(fq) + len(vq) + len(pq) + 192) * UNIT_NS
            pace = supply / n_skc_total
            deficit = 0.0

            def force_qk(sqt):
                # own-rep deadlines only: this rep's Q/K units for tile t_i
                # must be emitted by tile t_i; the next rep's units are
                # paced, never forced
                while (fq and fq[0] is not FENCE and fq[0][0] == st.rep_id
                       and fq[0][1] <= sqt):
                    fq.popleft()[3]()
            pending_tp = deque()  # (o_sb, hp, sq0) transposes, one pass late

            def emit_tp():
                while pending_tp:
                    o_sb, hp_, sq0_ = pending_tp.popleft()
                    for c in range(SQT // P):
                        tp = psum.tile([P, P], BF16, tag="mm", name="tp")
                        nc.tensor.transpose(tp[:], o_sb[:, c], id_sb[:])
                        nc.vector.tensor_copy(
                            oT[:, hp_, sq0_ + c * P : sq0_ + (c + 1) * P], tp[:]
                        )

            def drain():
                nonlocal deficit
                deficit = min(deficit, 8000.0)
                while deficit > 0.0:
                    if fq and fq[0] is FENCE:
                        fq.popleft()
                        continue
                    if vq:
                        _, c, fn = vq.popleft()
                    elif pq:
                        c, fn = pq.popleft()
                    elif fq:
                        _, _, c, fn = fq.popleft()
                    else:
                        deficit = min(deficit, 0.0)
                        return
                    fn()
                    deficit -= c

            def force_v(chunk):
                while vq and vq[0][0] <= chunk:
                    _, c, fn = vq.popleft()
                    fn()

            for sqt in range(n_sqt):
                sq0 = sqt * SQT
                nsk = (sq0 + SQT) // P
                force_qk(sqt)
                if sqt + 1 < n_sqt:
                    for sc8 in range(SQT // P):
                        vq.extend(v_sc_units(st, sqt + 1, sc8))
                for hp in range(HPC // 2):
                    po = [
                        opsum.tile([P, 4, 65], F32, tag=f"po{hi}", name=f"po{hi}")
                        for hi in range(2)
                    ]
                    pv_pending = None

                    def emit_pv(skc):
                        # PSUM start zeroes the whole 2KB bank (lazily): one
                        # start per hi-bank per pass; chunk regions init on
                        # their first (pending-zero) write, then accumulate.
                        force_v(skc + 1)  # one chunk of drain slack
                        for c in range(max(0, skc - 4 * sqt), SQT // P):
                            for hi in range(2):
                                nc.tensor.matmul(
                                    po[hi][:, c],
                                    et_by_skc[skc][:, hi, c * P : (c + 1) * P],
                                    st.v_sb[skc][:, 2 * hp + hi],
                                    start=(skc == 0 and c == 0),
                                    stop=(skc == nsk - 1 and c == SQT // P - 1),
                                )

                    deficit += PASS_BONUS
                    et_by_skc = {}
                    for skc in range(nsk):
                        off = max(0, skc * P - sq0)
                        wdt = SQT - off
                        ps_s = psum.tile([P, 2, SQT], F32, tag="score", name="ps_s")
                        for hi in range(2):
                            p0 = hi * DK
                            nc.tensor.matmul(
                                ps_s[:, hi, off:SQT],
                                kT[p0 : p0 + DK, hp, skc * P : (skc + 1) * P],
                                qT[p0 : p0 + DK, hp, sq0 + off : sq0 + SQT],
                                start=True,
                                stop=True,
                            )
                        if pv_pending is not None:
                            emit_pv(pv_pending)
                        if skc == 1:
                            emit_tp()  # prev pass's transposes (norm drained)
                            if hp == 0 and sqt >= 1:
                                # prev tile's out-projections: only queued
                                # once its last transposes are emitted above
                                for st_c in range((sqt - 1) * 4, sqt * 4):
                                    for nt in range(D // SQT):
                                        pq.extend(proj_units(oT, st_c, nt))
                        et = expp.tile([P, 2, SQT], BF16, tag="exp")
                        et_by_skc[skc] = et
                        nc.scalar.activation(
                            et[:, :, off:SQT],
                            ps_s[:, :, off:SQT],
                            mybir.ActivationFunctionType.Exp,
                            scale=float(1.0 / np.sqrt(DK)),
                        )
                        if skc * P >= sq0:  # diagonal chunk: mask invalid region
                            nc.vector.tensor_tensor(
                                et[:, :, off : off + P],
                                et[:, :, off : off + P],
                                ca_sb[:, None, :].to_broadcast([P, 2, P]),
                                mybir.AluOpType.mult,
                            )
                        deficit += pace
                        drain()
                        pv_pending = skc
                    emit_pv(pv_pending)

                    # normalize: o_sb[sq, c, hi, d] = po[sq, c, d] / den with
                    # den = po[:, c, 64] (the V ones column), per-partition
                    # scalars on DVE (GPSIMD cannot access PSUM); recip
                    # needs SBUF staging.
                    den = work.tile([P, 2, 4], F32, tag="den")
                    for hi in range(2):
                        nc.vector.tensor_copy(den[:, hi], po[hi][:, :, 64])
                    rec = work.tile([P, 2, 4], F32, tag="rec")
                    nc.vector.reciprocal_approx_fast(rec[:], den[:])
                    o_sb = work.tile([P, 4, 2, DK], BF16, tag="osb")
                    for c in range(SQT // P):
                        for hi in range(2):
                            nc.vector.tensor_scalar_mul(
                                o_sb[:, c, hi],
                                po[hi][:, c, 0:DK],
                                rec[:, hi, c : c + 1],
                            )
                    if sqt == 0 and hp == 0:
                        # prev-rep projections must finish before this rep's
                        # first oT write (same pool slot, emission-ordered)
                        while pq:
                            pq.popleft()[1]()
                    pending_tp.append((o_sb, hp, sq0))

            # flush remaining queued work
            emit_tp()
            while vq:
                vq.popleft()[2]()
            while pq:
                pq.popleft()[1]()
            while fq and fq[0] is not FENCE and fq[0][0] == st.rep_id:
                fq.popleft()[3]()  # own-rep stragglers (deadline-tagged)
            return fq

        def all_qk_units(st):
            """Flat unit list; each group's swap staggered 2 units later so
            its PE matmul never head-of-line-waits on the raw DVE cast."""
            u = deque()
            pend = []

            def push(unit):
                nonlocal pend
                keep = []
                for cd, su in pend:
                    if cd <= 0:
                        u.append(su)
                    else:
                        keep.append((cd - 1, su))
                pend = keep
                u.append(unit)

            for t_i in range(n_sqt):
                for name in ("wq", "wk"):
                    for dc in range(NDC):
                        g = qk_group_units(st, t_i, name, dc)
                        for unit in g[:-1]:
                            push(unit)
                        pend.append((4, g[-1]))
                # keep tags monotonic: a tile's last swaps must not cross
                # into the next tile's units (force_qk pops a tag-prefix)
                for _, su in pend:
                    u.append(su)
                pend = []
            return u

        cur = RepState()
        carry = all_qk_units(cur)
        # rep-0 prologue: only tile-0's Q/K groups run up front; the rest
        # drain inside rep 0's own attention under their tile deadlines
        need0 = sum(1 for e in carry if e[1] == 0)
        while need0:
            e = carry.popleft()
            e[3]()
            if e[1] == 0:
                need0 -= 1
        pending_proj = None
        for r in range(reps):
            pq = deque()
            if pending_proj is not None:
                # last tile's projections of rep r-1 (oT regions disjoint
                # from this rep's early transposes; drained as filler)
                for st_c in range((n_sqt - 1) * 4, n_sqt * 4):
                    for nt in range(D // SQT):
                        pq.extend(proj_units(*pending_proj, st_c, nt))
                pending_proj = None
            vq = deque()  # tile-0 V chunks (deadline: first PV groups)
            for sc8 in range(SQT // P):
                vq.extend(v_sc_units(cur, 0, sc8))
            nxt = RepState(r + 1) if r + 1 < reps else None
            fq = carry
            if nxt is not None:
                fq.append(FENCE)
                fq.extend(all_qk_units(nxt))
            carry = emit_attention(cur, fq, vq, pq)
            if carry and carry[0] is FENCE:
                carry.popleft()
            pending_proj = (cur.oT,)
            cur = nxt

        if pending_proj is not None:
            for st_c in range((n_sqt - 1) * 4, n_sqt * 4):
                for nt in range(D // SQT):
                    for c, fn in proj_units(*pending_proj, st_c, nt):
                        fn()

    nc.compile()
    return nc


# ---------------- host side ----------------

_ROPE_PERM = None


def _rope_perm():
    """Per-head de-interleave: even dims first, then odd dims."""
    global _ROPE_PERM
    if _ROPE_PERM is None:
        p = []
        for h in range(HPC):
            base = h * DK
            p += [base + 2 * k for k in range(DK // 2)]
            p += [base + 2 * k + 1 for k in range(DK // 2)]
        _ROPE_PERM = np.array(p)
    return _ROPE_PERM


def _bf16():
    import ml_dtypes

    return ml_dtypes.bfloat16


def _host_tables(token_positions, s):
    pos = np.asarray(token_positions).astype(np.float64)
    freqs = THETA ** (-np.arange(0, DK, 2, dtype=np.float64) / DK)  # [32]
    ang = pos[None, :] * freqs[:, None]  # [32, s]
    cos32 = np.cos(ang)
    sin32 = np.sin(ang)
    # layout [128, s]: per head block of 64: [cos32 (x1 half); cos32 (x2 half)]
    cos_t = np.empty((P, s), np.float32)
    sin_t = np.empty((P, s), np.float32)
    for h in range(2):  # 2 heads per 128-partition chunk
        b0 = h * DK
        cos_t[b0 : b0 + 32] = cos32
        cos_t[b0 + 32 : b0 + 64] = cos32
        sin_t[b0 : b0 + 32] = -sin32  # x1 half: -sin * x2
        sin_t[b0 + 32 : b0 + 64] = sin32  # x2 half: +sin * x1
    return cos_t, sin_t


_NC_CACHE = {}

# test harness hooks (off by default; harness calls kernel() directly)
TRACE = False
LAST = {}


def _get_program(b, s, reps=1):
    key = (b, s, reps)
    if key not in _NC_CACHE:
        _NC_CACHE[key] = build_program(b, s, reps)
    return _NC_CACHE[key]


def prepare_in_maps(x, Wq, Wk, Wv, Wo, token_positions):
    bf16 = _bf16()
    x = np.asarray(x, dtype=np.float32)
    Wq = np.asarray(Wq, dtype=np.float32)
    Wk = np.asarray(Wk, dtype=np.float32)
    Wv = np.asarray(Wv, dtype=np.float32)
    Wo = np.asarray(Wo, dtype=np.float32)
    b, s, _ = x.shape

    # [b, kc, p, s] transposed view of x
    xT = np.ascontiguousarray(x.transpose(0, 2, 1)).astype(bf16).reshape(
        b, NKC, P, s
    )
    cos_t, sin_t = _host_tables(token_positions, s)
    cos_t = cos_t.astype(bf16)
    sin_t = sin_t.astype(bf16)
    causal = np.triu(np.ones((P, P), np.float32)).astype(bf16)  # keep p <= f
    # swap permutation matrix (symmetric): swap(j) = j+-32 within each 64-block
    pm = np.zeros((P, P), np.float32)
    for h in range(2):
        b0 = h * DK
        for k in range(32):
            pm[b0 + k + 32, b0 + k] = 1.0
            pm[b0 + k, b0 + k + 32] = 1.0
    pm = pm.astype(bf16)

    perm = _rope_perm()
    in_maps = []
    for c in range(NCORES):
        bi, hh = c // 2, c % 2
        rows = slice(hh * BLK, (hh + 1) * BLK)
        wq_c = Wq[rows][perm]  # [512, D] rope-permuted rows
        wk_c = Wk[rows][perm]
        wv_c = Wv[rows]
        in_maps.append(
            {
                "xT": xT[bi],
                "wqT": np.ascontiguousarray(wq_c.T).astype(bf16).reshape(NKC, P, BLK),
                "wkT": np.ascontiguousarray(wk_c.T).astype(bf16).reshape(NKC, P, BLK),
                "wvT": np.ascontiguousarray(wv_c.T).astype(bf16).reshape(NKC, P, BLK),
                "woT": np.ascontiguousarray(Wo[:, rows].T)
                .astype(bf16)
                .reshape(NDC, P, D),
                "costab": cos_t,
                "sintab": sin_t,
                "pmswap": pm,
                "causal": causal,
                "identt": np.eye(P, dtype=np.float32).astype(bf16),
            }
        )

    return in_maps


def kernel(x, Wq, Wk, Wv, Wo, token_positions):
    b, s, _ = np.asarray(x).shape
    nc = _get_program(b, s)
    in_maps = prepare_in_maps(x, Wq, Wk, Wv, Wo, token_positions)
    res = run_bass_kernel_spmd(
        nc, in_maps, core_ids=list(range(NCORES)), trace=TRACE
    )
    LAST["exec_time_ns"] = res.exec_time_ns
    LAST["profile_json"] = res.profile_json
    out = np.empty((b, s, D), np.float32)
    for bi in range(b):
        out[bi] = res.results[2 * bi]["out"].astype(np.float32) + res.results[
            2 * bi + 1
        ]["out"].astype(np.float32)
    return out


# revision 90
# speedup vs baseline: 1.2024x; 1.2024x over previous
"""Multi-head self-attention with RoPE + causal mask on 8 Trainium2 NeuronCores.

Sharding: batch x head hybrid. Core c owns batch c//2 and head-half c%2
(8 of the 16 heads, a 512-wide slice of the QKV output dim / Wo input dim).
Each core computes a partial output out_c = O_c @ Wo_c^T of one batch; the
host sums the 2 partials per batch (the Wo row-split all-reduce done
host-side at gather time).

All matmul operands are bf16 (fp32 PSUM accumulation). The PE cost per
matmul is its OUTPUT free size (rows), so the kernel is organized to
minimize total rows:
  - Q^T/K^T produced in [dims, S] layout; RoPE de-interleave folded into
    the weight rows host-side; the +-32-partition swap via a PE
    permutation matmul; cos/sin combine on DVE.
  - V stored per 128-seq chunk as [128 seq, 8 heads, 65]: 64 dims + a
    ones column so the softmax denominator falls out of the P.V matmul.
  - ScoresT[sk, sq] = K^T.T @ Q^T per head pair (shared PSUM tile,
    (0,0)/(64,0) row tiles, K=64). exp on ScalarE from PSUM (1/8 scale
    folded; inputs bounded so no max subtraction). Causal masking
    multiplies only the 128x128 diagonal triangles.
  - P.V is TRANSPOSED: stationary = exp'd score chunk [sk, sq-128],
    moving = V chunk [sk, 65] -> po[sq, 65] accumulated over sk chunks.
    65 output rows per matmul instead of the sq-tile width: all 128
    output partitions carry sq, halving PV row cost (the matmuls
    stream at ~31ns; the 128-row FastWeightLoad hides under them).
    PSUM start zeroes a whole 2KB bank lazily, so one start per
    hi-bank per pass lets the 4 sq-chunk accumulations share a bank.
    den lands in column 64; normalization is a per-partition
    tensor_scalar on DVE; the [sq, d] -> [d, sq] fixup is a PE
    transpose (128 rows) + DVE drain, deferred one pass so it never
    head-of-line-waits on the norm.
  - The PE queue is strictly in-order and the attention phase is
    ACT(exp)-bound, so emission order IS the schedule: PV groups are
    emitted one k-chunk late, and ALL projection work (each rep's Q/K
    groups with per-tile deadlines, V chunks with per-chunk deadlines,
    out-projections one tile late) is split into SINGLE-MATMUL filler
    units paced evenly over the rep's 160 score-chunk iterations --
    the PE is the global bottleneck (~93% busy), so spreading the
    whole supply keeps ACT fully hidden and leaves no dead blob at
    rep boundaries. SBUF-only RoPE combine ops run on the otherwise
    idle Pool engine (GPSIMD cannot access PSUM).
"""

import sys

sys.path.insert(0, "/opt/trn_rl_repo")

import numpy as np
from collections import deque
from contextlib import ExitStack

import concourse.bass as bass
import concourse.tile as tile
from concourse import bacc, mybir
from concourse.bass_utils import run_bass_kernel_spmd

F32 = mybir.dt.float32
BF16 = mybir.dt.bfloat16

# problem constants (hardcoded per harness contract)
B = 4
S = 2048
D = 1024
NUM_HEADS = 16
DK = 64
THETA = 10000.0
NCORES = 8
HPC = 8  # heads per core
BLK = HPC * DK  # 512-wide per-core head-dim block
NDC = BLK // 128  # 4 dim chunks of 128
P = 128
SQT = 512  # sq tile width
NKC = D // P  # 8 contraction chunks for the projections

# pacing model (ns)
ROW_NS = 0.4166  # PE ns per output row at full clock
ACT_NS = 0.8333  # ACT ns per element per partition
ACT_FIX = 217.0  # ACT per-instruction fixed cost
PV_MM_NS = 45.0  # per 65-row PV matmul (rows + partially hidden LDW)
PASS_BONUS = 600.0  # extra filler at pass start (po ring-1 WAR cover)
UNIT_NS = SQT * ROW_NS  # nominal per-filler-unit PE cost


def build_program(b=B, s=S, reps=1):
    """Build the (SPMD-shared) per-core Bass program.

    reps>1 repeats the whole computation (for marginal-cost timing)."""
    nc = bacc.Bacc("TRN2", target_bir_lowering=False, debug=False)

    n_sqt = s // SQT  # sq tiles
    n_skc = s // P  # seq chunks of 128

    # ---- DRAM I/O ----
    xT = nc.dram_tensor("xT", [NKC, P, s], BF16, kind="ExternalInput").ap()
    wqT = nc.dram_tensor("wqT", [NKC, P, BLK], BF16, kind="ExternalInput").ap()
    wkT = nc.dram_tensor("wkT", [NKC, P, BLK], BF16, kind="ExternalInput").ap()
    wvT = nc.dram_tensor("wvT", [NKC, P, BLK], BF16, kind="ExternalInput").ap()
    woT = nc.dram_tensor("woT", [NDC, P, D], BF16, kind="ExternalInput").ap()
    costab = nc.dram_tensor("costab", [P, s], BF16, kind="ExternalInput").ap()
    sintab = nc.dram_tensor("sintab", [P, s], BF16, kind="ExternalInput").ap()
    pmswap = nc.dram_tensor("pmswap", [P, P], BF16, kind="ExternalInput").ap()
    causal = nc.dram_tensor("causal", [P, P], BF16, kind="ExternalInput").ap()
    identt = nc.dram_tensor("identt", [P, P], BF16, kind="ExternalInput").ap()
    out = nc.dram_tensor("out", [s, D], BF16, kind="ExternalOutput").ap()

    FENCE = ("fence",)  # separates one rep's Q/K units from the next's

    with tile.TileContext(nc) as tc, ExitStack() as ctx:
        consts = ctx.enter_context(tc.tile_pool(name="consts", bufs=1))
        # xt tiles stay alive from the (interleaved, early) Q/K groups until
        # the V projections of the owning rep: ring of 5 gives the cross-rep
        # WAR an extra tile of slack.
        xpool = ctx.enter_context(tc.tile_pool(name="xpool", bufs=5))
        big = ctx.enter_context(tc.tile_pool(name="big", bufs=1))
        # qT/kT double-buffered: the next rep's Q/K groups (written during
        # this rep's attention) must not clobber the tiles attention reads.
        qkp = ctx.enter_context(tc.tile_pool(name="qkp", bufs=2))
        work = ctx.enter_context(tc.tile_pool(name="work", bufs=2))
        expp = ctx.enter_context(tc.tile_pool(name="expp", bufs=4))
        psum = ctx.enter_context(tc.tile_pool(name="psum", bufs=2, space="PSUM"))
        opsum = ctx.enter_context(tc.tile_pool(name="opsum", bufs=1, space="PSUM"))

        # ---- constants resident in SBUF ----
        w_sb = {}
        for name, ap in (("wq", wqT), ("wk", wkT), ("wv", wvT)):
            t = consts.tile([P, NKC, BLK], BF16, tag=f"w_{name}")
            for kc in range(NKC):
                nc.sync.dma_start(t[:, kc], ap[kc])
            w_sb[name] = t
        wo_sb = consts.tile([P, NDC, D], BF16, tag="wo")
        for dc in range(NDC):
            nc.sync.dma_start(wo_sb[:, dc], woT[dc])
        cos_sb = consts.tile([P, s], BF16, tag="cos")
        nc.sync.dma_start(cos_sb[:], costab)
        sin_sb = consts.tile([P, s], BF16, tag="sin")
        nc.sync.dma_start(sin_sb[:], sintab)
        pm_sb = consts.tile([P, P], BF16, tag="pm")
        nc.sync.dma_start(pm_sb[:], pmswap)
        ca_sb = consts.tile([P, P], BF16, tag="causal")
        nc.sync.dma_start(ca_sb[:], causal)
        id_sb = consts.tile([P, P], BF16, tag="ident")
        nc.sync.dma_start(id_sb[:], identt)

        class RepState:
            def __init__(self, rep_id=0):
                self.rep_id = rep_id
                self.qT = qkp.tile([P, NDC, s], BF16, tag="qT", name="qT")
                self.kT = qkp.tile([P, NDC, s], BF16, tag="kT", name="kT")
                self.oT = None  # allocated at attention time
                self.xts = {}  # t_i -> list of 8 xt tiles (shared Q/K/V)
                self.v_sb = None  # allocated at the first V unit

        def get_xt(st, t_i):
            if t_i not in st.xts:
                w = slice(t_i * SQT, (t_i + 1) * SQT)
                xt = []
                for kc in range(NKC):
                    xtc = xpool.tile([P, SQT], BF16, tag=f"xt{kc}", name=f"xt{kc}")
                    nc.sync.dma_start(xtc[:], xT[kc, :, w])
                    xt.append(xtc)
                st.xts[t_i] = xt
            return st.xts[t_i]

        # ---------- filler units: (pe_cost_ns, emit_fn) ----------

        def qk_group_units(st, t_i, name, dc):
            """One Q or K projection group as 9 single-matmul units."""
            w = slice(t_i * SQT, (t_i + 1) * SQT)
            dst = st.qT if name == "wq" else st.kT
            box = {}

            def mm(kc):
                def emit():
                    if kc == 0:
                        box["xt"] = get_xt(st, t_i)
                        box["ps"] = psum.tile([P, SQT], F32, tag="mm", name="ps")
                    nc.tensor.matmul(
                        box["ps"][:],
                        w_sb[name][:, kc, dc * P : (dc + 1) * P],
                        box["xt"][kc][:],
                        start=(kc == 0),
                        stop=(kc == NKC - 1),
                    )
                    if kc == NKC - 1:
                        raw = work.tile([P, SQT], BF16, tag="raw")
                        # ACT Copy (same act table as Exp, no switch cost):
                        # starts right when mm7 drains instead of queueing
                        # behind DVE filler work -- the swap matmul and the
                        # next group's mm-ring WAR both wait on this cast
                        nc.scalar.activation(
                            raw[:], box["ps"][:],
                            mybir.ActivationFunctionType.Copy,
                        )
                        box["raw"] = raw

                return emit

            def swap():
                raw = box["raw"]
                ps_sw = psum.tile([P, SQT], F32, tag="mm", name="ps_sw")
                nc.tensor.matmul(ps_sw[:], pm_sb[:], raw[:], start=True, stop=True)
                # tco and the final add are SBUF-only: run them on the idle
                # Pool engine, halving this chain's DVE queue footprint
                tco = work.tile([P, SQT], BF16, tag="tco")
                nc.gpsimd.tensor_tensor(
                    tco[:], raw[:], cos_sb[:, w], mybir.AluOpType.mult
                )
                tsi = work.tile([P, SQT], BF16, tag="tsi")
                nc.vector.tensor_tensor(
                    tsi[:], ps_sw[:], sin_sb[:, w], mybir.AluOpType.mult
                )
                nc.gpsimd.tensor_tensor(
                    dst[:, dc, w], tco[:], tsi[:], mybir.AluOpType.add
                )

            units = [(st.rep_id, t_i, SQT * ROW_NS, mm(kc)) for kc in range(NKC)]
            units.append((st.rep_id, t_i, SQT * ROW_NS, swap))
            return units

        def v_sc_units(st, t_i, sc8):
            """V projection for one 128-seq chunk as 8 units, tagged with the
            chunk index so PV emission can force-drain them in time."""
            if st.v_sb is None:
                st.v_sb = [
                    big.tile([P, HPC, 65], BF16, tag=f"v{sc}", name=f"v{sc}")
                    for sc in range(n_skc)
                ]
            sc = t_i * (SQT // P) + sc8
            box = {}

            def mm(kc):
                def emit():
                    if kc == 0:
                        box["xt"] = get_xt(st, t_i)
                        box["ps"] = psum.tile([P, SQT], F32, tag="mm", name="ps_v")
                    nc.tensor.matmul(
                        box["ps"][:],
                        box["xt"][kc][:, sc8 * P : (sc8 + 1) * P],
                        w_sb["wv"][:, kc],
                        start=(kc == 0),
                        stop=(kc == NKC - 1),
                    )
                    if kc == NKC - 1:
                        vt = st.v_sb[sc]
                        nc.gpsimd.memset(vt[:, :, DK:65], 1.0)
                        # ACT Copy (shares Exp's act table): starts when the
                        # projection drains; the PV moving read gates on it
                        nc.scalar.activation(
                            vt[:, :, 0:DK], box["ps"][:],
                            mybir.ActivationFunctionType.Copy,
                        )

                return emit

            return [(sc, SQT * ROW_NS, mm(kc)) for kc in range(NKC)]

        def proj_units(oT_src, st_c, nt):
            """One out-projection group (4 accumulating matmuls + drain)."""
            box = {}

            def mm(dc):
                def emit():
                    if dc == 0:
                        box["ps"] = psum.tile([P, SQT], F32, tag="mm", name="ps_p")
                    nc.tensor.matmul(
                        box["ps"][:],
                        oT_src[:, dc, st_c * P : (st_c + 1) * P],
                        wo_sb[:, dc, nt * SQT : (nt + 1) * SQT],
                        start=(dc == 0),
                        stop=(dc == NDC - 1),
                    )
                    if dc == NDC - 1:
                        ob = work.tile([P, SQT], BF16, tag="ob")
                        nc.vector.tensor_copy(ob[:], box["ps"][:])
                        nc.sync.dma_start(
                            out[
                                st_c * P : (st_c + 1) * P,
                                nt * SQT : (nt + 1) * SQT,
                            ],
                            ob[:],
                        )

                return emit

            return [(SQT * ROW_NS, mm(dc)) for dc in range(NDC)]

        # ---------- attention ----------

        def emit_attention(st, fq, vq, pq):
            """fq: deque of (cost, fn) filler units (next rep's Q/K groups).
            vq: deque of (chunk, cost, fn) V units with chunk deadlines,
            seeded with tile-0 chunks. pq: deque of out-projection units,
            seeded with the previous rep's last tile."""
            st.oT = big.tile([P, NDC, s], BF16, tag="oT", name="oT")
            qT, kT, oT = st.qT, st.kT, st.oT
            # pace the whole rep's filler supply (next rep's Q/K groups, V
            # chunks, out-projections) evenly over the 160 skc iterations:
            # the PE is the global bottleneck, so spreading ALL of it through
            # the ACT-bound attention keeps ACT fully hidden and leaves no
            # dead blob at the rep boundary.
            n_skc_total = sum(4 * ((t + 1) * SQT // P) for t in range(n_sqt))
            supply = (len(fq) + len(vq) + len(pq) + 192) * UNIT_NS
            pace = supply / n_skc_total
            deficit = 0.0

            def force_qk(sqt):
                # own-rep deadlines only: this rep's Q/K units for tile t_i
                # must be emitted by tile t_i; the next rep's units are
                # paced, never forced
                while (fq and fq[0] is not FENCE and fq[0][0] == st.rep_id
                       and fq[0][1] <= sqt):
                    fq.popleft()[3]()
            pending_tp = deque()  # (o_sb, hp, sq0) transposes, one pass late

            def emit_tp():
                while pending_tp:
                    o_sb, hp_, sq0_ = pending_tp.popleft()
                    for c in range(SQT // P):
                        tp = psum.tile([P, P], BF16, tag="mm", name="tp")
                        nc.tensor.transpose(tp[:], o_sb[:, c], id_sb[:])
                        nc.vector.tensor_copy(
                            oT[:, hp_, sq0_ + c * P : sq0_ + (c + 1) * P], tp[:]
                        )

            def drain():
                nonlocal deficit
                deficit = min(deficit, 8000.0)
                while deficit > 0.0:
                    if fq and fq[0] is FENCE:
                        fq.popleft()
                        continue
                    if vq:
                        _, c, fn = vq.popleft()
                    elif pq:
                        c, fn = pq.popleft()
                    elif fq:
                        _, _, c, fn = fq.popleft()
                    else:
                        deficit = min(deficit, 0.0)
                        return
                    fn()
                    deficit -= c

            def force_v(chunk):
                while vq and vq[0][0] <= chunk:
                    _, c, fn = vq.popleft()
                    fn()

            for sqt in range(n_sqt):
                sq0 = sqt * SQT
                nsk = (sq0 + SQT) // P
                force_qk(sqt)
                if sqt + 1 < n_sqt:
                    for sc8 in range(SQT // P):
                        vq.extend(v_sc_units(st, sqt + 1, sc8))
                for hp in range(HPC // 2):
                    po = [
                        opsum.tile([P, 4, 65], F32, tag=f"po{hi}", name=f"po{hi}")
                        for hi in range(2)
                    ]
                    pend_pv = deque()

                    def emit_pv(skc):
                        # PSUM start zeroes the whole 2KB bank (lazily): one
                        # start per hi-bank per pass; chunk regions init on
                        # their first (pending-zero) write, then accumulate.
                        force_v(skc + 1)  # one chunk of drain slack
                        for c in range(max(0, skc - 4 * sqt), SQT // P):
                            for hi in range(2):
                                nc.tensor.matmul(
                                    po[hi][:, c],
                                    et_by_skc[skc][:, hi, c * P : (c + 1) * P],
                                    st.v_sb[skc][:, 2 * hp + hi],
                                    start=(skc == 0 and c == 0),
                                    stop=(skc == nsk - 1 and c == SQT // P - 1),
                                )

                    deficit += PASS_BONUS
                    et_by_skc = {}
                    for skc in range(nsk):
                        off = max(0, skc * P - sq0)
                        wdt = SQT - off
                        ps_s = psum.tile([P, 2, SQT], F32, tag="score", name="ps_s")
                        for hi in range(2):
                            p0 = hi * DK
                            nc.tensor.matmul(
                                ps_s[:, hi, off:SQT],
                                kT[p0 : p0 + DK, hp, skc * P : (skc + 1) * P],
                                qT[p0 : p0 + DK, hp, sq0 + off : sq0 + SQT],
                                start=True,
                                stop=True,
                            )
                        # non-diag PV one skc late; DIAG PV two late -- its
                        # et stationary gates on the causal mask (DVE), and
                        # the extra skc of cover absorbs that latency (the
                        # et ring of 4 affords the longer lifetime)
                        while pend_pv and skc - pend_pv[0] >= (
                            2 if pend_pv[0] >= 4 * sqt else 1
                        ):
                            emit_pv(pend_pv.popleft())
                        if skc == 1:
                            emit_tp()  # prev pass's transposes (norm drained)
                            if hp == 0 and sqt >= 1:
                                # prev tile's out-projections: only queued
                                # once its last transposes are emitted above
                                for st_c in range((sqt - 1) * 4, sqt * 4):
                                    for nt in range(D // SQT):
                                        pq.extend(proj_units(oT, st_c, nt))
                        et = expp.tile([P, 2, SQT], BF16, tag="exp")
                        et_by_skc[skc] = et
                        nc.scalar.activation(
                            et[:, :, off:SQT],
                            ps_s[:, :, off:SQT],
                            mybir.ActivationFunctionType.Exp,
                            scale=float(1.0 / np.sqrt(DK)),
                        )
                        if skc * P >= sq0:  # diagonal chunk: mask invalid region
                            nc.vector.tensor_tensor(
                                et[:, :, off : off + P],
                                et[:, :, off : off + P],
                                ca_sb[:, None, :].to_broadcast([P, 2, P]),
                                mybir.AluOpType.mult,
                            )
                        deficit += pace
                        drain()
                        pend_pv.append(skc)
                    while pend_pv:
                        emit_pv(pend_pv.popleft())

                    # normalize: o_sb[sq, c, hi, d] = po[sq, c, d] / den with
                    # den = po[:, c, 64] (the V ones column), per-partition
                    # scalars on DVE (GPSIMD cannot access PSUM); recip
                    # needs SBUF staging.
                    den = work.tile([P, 2, 4], F32, tag="den")
                    for hi in range(2):
                        nc.vector.tensor_copy(den[:, hi], po[hi][:, :, 64])
                    rec = work.tile([P, 2, 4], F32, tag="rec")
                    nc.vector.reciprocal_approx_fast(rec[:], den[:])
                    o_sb = work.tile([P, 4, 2, DK], BF16, tag="osb")
                    for c in range(SQT // P):
                        for hi in range(2):
                            nc.vector.tensor_scalar_mul(
                                o_sb[:, c, hi],
                                po[hi][:, c, 0:DK],
                                rec[:, hi, c : c + 1],
                            )
                    if sqt == 0 and hp == 0:
                        # prev-rep projections must finish before this rep's
                        # first oT write (same pool slot, emission-ordered)
                        while pq:
                            pq.popleft()[1]()
                    pending_tp.append((o_sb, hp, sq0))

            # flush remaining queued work
            emit_tp()
            while vq:
                vq.popleft()[2]()
            while pq:
                pq.popleft()[1]()
            while fq and fq[0] is not FENCE and fq[0][0] == st.rep_id:
                fq.popleft()[3]()  # own-rep stragglers (deadline-tagged)
            return fq

        def all_qk_units(st):
            """Flat unit list; each group's swap staggered 2 units later so
            its PE matmul never head-of-line-waits on the raw DVE cast."""
            u = deque()
            pend = []

            def push(unit):
                nonlocal pend
                keep = []
                for cd, su in pend:
                    if cd <= 0:
                        u.append(su)
                    else:
                        keep.append((cd - 1, su))
                pend = keep
                u.append(unit)

            for t_i in range(n_sqt):
                for name in ("wq", "wk"):
                    for dc in range(NDC):
                        g = qk_group_units(st, t_i, name, dc)
                        for unit in g[:-1]:
                            push(unit)
                        pend.append((4, g[-1]))
                # keep tags monotonic: a tile's last swaps must not cross
                # into the next tile's units (force_qk pops a tag-prefix)
                for _, su in pend:
                    u.append(su)
                pend = []
            return u

        cur = RepState()
        carry = all_qk_units(cur)
        # rep-0 prologue: only tile-0's Q/K groups run up front; the rest
        # drain inside rep 0's own attention under their tile deadlines
        need0 = sum(1 for e in carry if e[1] == 0)
        while need0:
            e = carry.popleft()
            e[3]()
            if e[1] == 0:
                need0 -= 1
        pending_proj = None
        for r in range(reps):
            pq = deque()
            if pending_proj is not None:
                # last tile's projections of rep r-1 (oT regions disjoint
                # from this rep's early transposes; drained as filler)
                for st_c in range((n_sqt - 1) * 4, n_sqt * 4):
                    for nt in range(D // SQT):
                        pq.extend(proj_units(*pending_proj, st_c, nt))
                pending_proj = None
            vq = deque()  # tile-0 V chunks (deadline: first PV groups)
            for sc8 in range(SQT // P):
                vq.extend(v_sc_units(cur, 0, sc8))
            nxt = RepState(r + 1) if r + 1 < reps else None
            fq = carry
            if nxt is not None:
                fq.append(FENCE)
                fq.extend(all_qk_units(nxt))
            carry = emit_attention(cur, fq, vq, pq)
            if carry and carry[0] is FENCE:
                carry.popleft()
            pending_proj = (cur.oT,)
            cur = nxt

        if pending_proj is not None:
            for st_c in range((n_sqt - 1) * 4, n_sqt * 4):
                for nt in range(D // SQT):
                    for c, fn in proj_units(*pending_proj, st_c, nt):
                        fn()

    nc.compile()
    return nc


# ---------------- host side ----------------

_ROPE_PERM = None


def _rope_perm():
    """Per-head de-interleave: even dims first, then odd dims."""
    global _ROPE_PERM
    if _ROPE_PERM is None:
        p = []
        for h in range(HPC):
            base = h * DK
            p += [base + 2 * k for k in range(DK // 2)]
            p += [base + 2 * k + 1 for k in range(DK // 2)]
        _ROPE_PERM = np.array(p)
    return _ROPE_PERM


def _bf16():
    import ml_dtypes

    return ml_dtypes.bfloat16


def _host_tables(token_positions, s):
    pos = np.asarray(token_positions).astype(np.float64)
    freqs = THETA ** (-np.arange(0, DK, 2, dtype=np.float64) / DK)  # [32]
    ang = pos[None, :] * freqs[:, None]  # [32, s]
    cos32 = np.cos(ang)
    sin32 = np.sin(ang)
    # layout [128, s]: per head block of 64: [cos32 (x1 half); cos32 (x2 half)]
    cos_t = np.empty((P, s), np.float32)
    sin_t = np.empty((P, s), np.float32)
    for h in range(2):  # 2 heads per 128-partition chunk
        b0 = h * DK
        cos_t[b0 : b0 + 32] = cos32
        cos_t[b0 + 32 : b0 + 64] = cos32
        sin_t[b0 : b0 + 32] = -sin32  # x1 half: -sin * x2
        sin_t[b0 + 32 : b0 + 64] = sin32  # x2 half: +sin * x1
    return cos_t, sin_t


_NC_CACHE = {}

# test harness hooks (off by default; harness calls kernel() directly)
TRACE = False
LAST = {}


def _get_program(b, s, reps=1):
    key = (b, s, reps)
    if key not in _NC_CACHE:
        _NC_CACHE[key] = build_program(b, s, reps)
    return _NC_CACHE[key]


def prepare_in_maps(x, Wq, Wk, Wv, Wo, token_positions):
    bf16 = _bf16()
    x = np.asarray(x, dtype=np.float32)
    Wq = np.asarray(Wq, dtype=np.float32)
    Wk = np.asarray(Wk, dtype=np.float32)
    Wv = np.asarray(Wv, dtype=np.float32)
    Wo = np.asarray(Wo, dtype=np.float32)
    b, s, _ = x.shape

    # [b, kc, p, s] transposed view of x
    xT = np.ascontiguousarray(x.transpose(0, 2, 1)).astype(bf16).reshape(
        b, NKC, P, s
    )
    cos_t, sin_t = _host_tables(token_positions, s)
    cos_t = cos_t.astype(bf16)
    sin_t = sin_t.astype(bf16)
    causal = np.triu(np.ones((P, P), np.float32)).astype(bf16)  # keep p <= f
    # swap permutation matrix (symmetric): swap(j) = j+-32 within each 64-block
    pm = np.zeros((P, P), np.float32)
    for h in range(2):
        b0 = h * DK
        for k in range(32):
            pm[b0 + k + 32, b0 + k] = 1.0
            pm[b0 + k, b0 + k + 32] = 1.0
    pm = pm.astype(bf16)

    perm = _rope_perm()
    in_maps = []
    for c in range(NCORES):
        bi, hh = c // 2, c % 2
        rows = slice(hh * BLK, (hh + 1) * BLK)
        wq_c = Wq[rows][perm]  # [512, D] rope-permuted rows
        wk_c = Wk[rows][perm]
        wv_c = Wv[rows]
        in_maps.append(
            {
                "xT": xT[bi],
                "wqT": np.ascontiguousarray(wq_c.T).astype(bf16).reshape(NKC, P, BLK),
                "wkT": np.ascontiguousarray(wk_c.T).astype(bf16).reshape(NKC, P, BLK),
                "wvT": np.ascontiguousarray(wv_c.T).astype(bf16).reshape(NKC, P, BLK),
                "woT": np.ascontiguousarray(Wo[:, rows].T)
                .astype(bf16)
                .reshape(NDC, P, D),
                "costab": cos_t,
                "sintab": sin_t,
                "pmswap": pm,
                "causal": causal,
                "identt": np.eye(P, dtype=np.float32).astype(bf16),
            }
        )

    return in_maps


def kernel(x, Wq, Wk, Wv, Wo, token_positions):
    b, s, _ = np.asarray(x).shape
    nc = _get_program(b, s)
    in_maps = prepare_in_maps(x, Wq, Wk, Wv, Wo, token_positions)
    res = run_bass_kernel_spmd(
        nc, in_maps, core_ids=list(range(NCORES)), trace=TRACE
    )
    LAST["exec_time_ns"] = res.exec_time_ns
    LAST["profile_json"] = res.profile_json
    out = np.empty((b, s, D), np.float32)
    for bi in range(b):
        out[bi] = res.results[2 * bi]["out"].astype(np.float32) + res.results[
            2 * bi + 1
        ]["out"].astype(np.float32)
    return out
